# revision 17
# baseline (speedup 1.0000x reference)
"""Trainium2 Bass kernel for nn_ContextualPositionEmbedding (B,H,S,D,NPOS = 2,16,2048,64,64).

out[b,h,i,j] = logits + interp(logits_int, pos) where
  gates = sigmoid(attn_logits + log(mask));  pos = clip(reverse-cumsum_j(gates), max 63)
  logits_int = query @ pos_emb;  interp = linear interpolation of logits_int at pos.

Sharding: batch*heads (32 pairs) split 4-per-core across 8 NeuronCores; each core
processes 64 independent [128 x 2048] row-tiles. pos_emb replicated. No collectives.
I/O in fp16 (tolerance is 2e-2 relative; fp16 rounding is far inside budget).

Algorithm per tile (exact where the validity flags pass; a host-side numpy fallback
covers anything else — never triggered for the target workload):
  - pos saturates at 63 for all but the last WS=160 key columns (flag-checked), so
    out = logits + f[r,63] there: one full-row scalar-engine pass with per-row bias.
  - strip [JCUT, 2048): pos = min(reverse-cumsum, 63) comes from ONE reversed
    tensor_tensor_scan with in-scan min-clip. floor(pos) via the +2^23 round trick
    plus an is_gt correction. floor(pos) is a staircase crossing each level 63..1
    exactly once; the last column per level is found with a duplicate-index
    local_scatter (HW is last-wins), then the fp16 table deltas are scattered to
    those columns and prefix-summed (fp32 state): S1[j] = f[fl_j] - f[63] and
    S2[j] = -nd[fl_j] = f[cl_j] - f[fl_j]. out_strip += S1 + w*S2.
"""

import numpy as np
from contextlib import ExitStack

import concourse.bass as bass
import concourse.tile as tile
from concourse import bacc, mybir
from concourse.bass_utils import run_bass_kernel_spmd

F32 = mybir.dt.float32
F16 = mybir.dt.float16
I32 = mybir.dt.int32
I16 = mybir.dt.int16
AF = mybir.ActivationFunctionType
OP = mybir.AluOpType

B, H, S, D, NPOS = 2, 16, 2048, 64, 64
N_CORES = 8
JCUT = 1888
WS = S - JCUT            # 160-wide exact strip
BH = B * H               # 32
BH_PER_CORE = BH // N_CORES   # 4
RB = S // 128            # 16 row-blocks per (b,h)
NT = BH_PER_CORE * RB    # 64 tiles per core
BIG = 8388608.0          # 2^23: float->rne(int) trick
DB = 2 * WS + 2          # dbuf width


def build_program(ntiles=NT, dbg=False):
    nc = bacc.Bacc("TRN2", target_bir_lowering=False, debug=False)
    attn = nc.dram_tensor("attn", [ntiles, 128, S], F16, kind="ExternalInput")
    qT = nc.dram_tensor("qT", [64, ntiles * 128], F32, kind="ExternalInput")
    pe = nc.dram_tensor("pe", [D, NPOS], F32, kind="ExternalInput")
    iota = nc.dram_tensor("iota", [128, WS], I16, kind="ExternalInput")
    out = nc.dram_tensor("out", [ntiles, 128, S], F16, kind="ExternalOutput")
    flags = nc.dram_tensor("flags", [128, ntiles], F32, kind="ExternalOutput")
    if dbg:
        dbg_t = {
            name: nc.dram_tensor(f"dbg_{name}", [ntiles, 128, width], dt,
                                 kind="ExternalOutput")
            for name, width, dt in [
                ("pos", WS, F32), ("w", WS, F32), ("idx1", WS, I16),
                ("idx2", 128, I16), ("dbuf", DB, F16), ("S1", WS, F32),
                ("S2", WS, F32), ("data2", 128, F16), ("f", NPOS, F32),
                ("h", WS, F32),
            ]
        }

    with tile.TileContext(nc) as tc, ExitStack() as ctx:
        const_pool = ctx.enter_context(tc.tile_pool(name="const", bufs=1))
        big_pool = ctx.enter_context(tc.tile_pool(name="big", bufs=6))
        q_pool = ctx.enter_context(tc.tile_pool(name="q", bufs=4))
        psum_pool = ctx.enter_context(tc.tile_pool(name="ps", bufs=4, space="PSUM"))
        tb_pool = ctx.enter_context(tc.tile_pool(name="tb", bufs=6))
        s_pool = ctx.enter_context(tc.tile_pool(name="s", bufs=6))

        pe_sb = const_pool.tile([64, NPOS], F32)
        nc.sync.dma_start(pe_sb[:], pe.ap())
        iota_sb = const_pool.tile([128, WS], I16)
        nc.sync.dma_start(iota_sb[:], iota.ap())
        zi16 = const_pool.tile([128, WS], I16)
        nc.vector.memset(zi16[:], 0)
        c63 = const_pool.tile([128, WS], F32)
        nc.vector.memset(c63[:], 63.0)
        flags_sb = const_pool.tile([128, ntiles], F32)
        # all per-tile query blocks in one DMA: [64, ntiles*128]
        qt_all = const_pool.tile([64, ntiles * 128], F32)
        nc.sync.dma_start(qt_all[:], qT.ap())

        for t in range(ntiles):
            # ---- load (fp16 logits)
            lg = big_pool.tile([128, S], F16, tag="lg")
            nc.sync.dma_start(lg[:], attn.ap()[t])

            # ---- table f = q @ pos_emb  [128, 64]
            fps = psum_pool.tile([128, NPOS], F32, tag="fps")
            nc.tensor.matmul(fps[:], lhsT=qt_all[:, t * 128:(t + 1) * 128],
                             rhs=pe_sb[:], start=True, stop=True)
            f = tb_pool.tile([128, NPOS], F32, tag="f")
            nc.scalar.activation(f[:], fps[:], AF.Copy)

            # ---- fp16 deltas: data2[:,n]=f[n]-f[n+1]; data2[:,64+n]=nd[n+1]-nd[n]
            data2 = tb_pool.tile([128, 128], F16, tag="data2")
            nc.vector.memset(data2[:, 63:64], 0.0)
            nc.vector.tensor_sub(data2[:, 0:63], f[:, 0:63], f[:, 1:64])
            nc.vector.tensor_sub(data2[:, 64:127], data2[:, 1:64], data2[:, 0:63])
            nc.vector.memset(data2[:, 127:128], 0.0)

            # ---- strip: pos = min(reverse-cumsum(sigmoid(lg)), 63), one scan
            gp = s_pool.tile([128, WS], F32, tag="gp")
            nc.scalar.activation(gp[:], lg[:, JCUT:S], AF.Sigmoid)
            pos = s_pool.tile([128, WS], F32, tag="pos")
            nc.vector.tensor_tensor_scan(pos[:, ::-1], gp[:, ::-1], c63[:],
                                         0.0, OP.add, OP.min)

            # fl = floor(pos) via rne (+2^23 trick) + correction; w = pos - fl
            fp_ = s_pool.tile([128, WS], F32, tag="fp")
            nc.vector.tensor_scalar(fp_[:], pos[:], BIG, -BIG, OP.add, OP.add)
            d = s_pool.tile([128, WS], F32, tag="d")
            nc.vector.tensor_sub(d[:], fp_[:], pos[:])
            corr = s_pool.tile([128, WS], F32, tag="corr")
            nc.vector.tensor_scalar(corr[:], d[:], 0.0, None, OP.is_gt)
            w = s_pool.tile([128, WS], F32, tag="w")
            nc.vector.tensor_sub(w[:], corr[:], d[:])
            idx1 = s_pool.tile([128, WS], I16, tag="idx1")
            nc.vector.scalar_tensor_tensor(idx1[:], fp_[:], -1.0, corr[:],
                                           OP.add, OP.subtract)

            # ---- scatter 1 (dup last-wins): last column per level -> idx2[:,0:64]
            idx2 = s_pool.tile([128, 128], I16, tag="idx2")
            nc.gpsimd.local_scatter(idx2[:, 0:64], iota_sb[:], idx1[:],
                                    channels=128, num_elems=64, num_idxs=WS)
            nc.gpsimd.tensor_scalar(idx2[:, 64:127], idx2[:, 0:63], float(WS + 1),
                                    None, OP.add)
            nc.gpsimd.memset(idx2[:, 127:128], -1)
            # ---- scatter 2: fp16 deltas to (drop column + 1), per channel
            dbuf = s_pool.tile([128, DB], F16, tag="dbuf")
            nc.gpsimd.local_scatter(dbuf[:], data2[:], idx2[:],
                                    channels=128, num_elems=DB, num_idxs=128)

            # ---- inclusive prefix sums (fp32 state over fp16)
            S1 = s_pool.tile([128, WS], F32, tag="S1")
            nc.vector.tensor_tensor_scan(S1[:], dbuf[:, 0:WS], zi16[:],
                                         0.0, OP.add, OP.add)
            S2 = s_pool.tile([128, WS], F32, tag="S2")
            nc.vector.tensor_tensor_scan(S2[:], dbuf[:, WS + 1:2 * WS + 1], zi16[:],
                                         0.0, OP.add, OP.add)
            wg2 = s_pool.tile([128, WS], F32, tag="wg2")
            nc.vector.tensor_mul(wg2[:], w[:], S2[:])
            h = s_pool.tile([128, WS], F32, tag="h")
            nc.vector.tensor_add(h[:], S1[:], wg2[:])

            # ---- full-row bias pass: lg += f[:,63]  (in-place on lg)
            nc.scalar.activation(lg[:], lg[:], AF.Identity,
                                 bias=f[:, 63:64], scale=1.0)
            # ---- strip combine
            nc.gpsimd.tensor_add(lg[:, JCUT:S], lg[:, JCUT:S], h[:])

            # ---- flags: pos[0] == 63 (saturated) AND all levels 1..63 deposited
            posok = s_pool.tile([128, 1], F32, tag="posok")
            nc.vector.tensor_scalar(posok[:], pos[:, 0:1], 63.0, None, OP.is_ge)
            rmin = s_pool.tile([128, 1], I16, tag="rmin")
            nc.vector.tensor_reduce(rmin[:], idx2[:, 0:63], mybir.AxisListType.X,
                                    OP.min)
            nc.vector.scalar_tensor_tensor(flags_sb[:, t:t + 1], rmin[:], 0.5,
                                           posok[:], OP.is_ge, OP.mult)

            if dbg:
                for name, ap_ in [("pos", pos[:]), ("w", w[:]), ("idx1", idx1[:]),
                                  ("idx2", idx2[:]), ("dbuf", dbuf[:]),
                                  ("S1", S1[:]), ("S2", S2[:]),
                                  ("data2", data2[:]), ("f", f[:]), ("h", h[:])]:
                    nc.sync.dma_start(dbg_t[name].ap()[t], ap_)

            # ---- store (Pool SWDGE queue: triggered right after the strip add,
            # same engine, so it never head-blocks another engine's sequencer)
            nc.gpsimd.dma_start(out.ap()[t], lg[:])

        nc.sync.dma_start(flags.ap(), flags_sb[:])

    nc.compile()
    return nc


_PROG_CACHE = {}


def _get_program(ntiles=NT):
    if ntiles not in _PROG_CACHE:
        _PROG_CACHE[ntiles] = build_program(ntiles)
    return _PROG_CACHE[ntiles]


def _prep_core_inputs(attn_f16, qT_all, pe2d, iota_np):
    """attn_f16: [BH, S, S] fp16; qT_all: [BH, D, S]. Returns list of 8 in_maps."""
    in_maps = []
    for c in range(N_CORES):
        sl = slice(c * BH_PER_CORE, (c + 1) * BH_PER_CORE)
        a = attn_f16[sl].reshape(NT, 128, S)
        q = np.ascontiguousarray(
            qT_all[sl].reshape(BH_PER_CORE, D, RB, 128).transpose(1, 0, 2, 3)
        ).reshape(D, NT * 128)
        in_maps.append({"attn": np.ascontiguousarray(a), "qT": q,
                        "pe": pe2d, "iota": iota_np})
    return in_maps


def _reference_fallback(query, attn_logits, mask, pos_emb):
    logits = attn_logits + np.log(mask)
    gates = 1.0 / (1.0 + np.exp(-logits))
    pos = np.cumsum(gates[..., ::-1], axis=-1)[..., ::-1]
    pos = np.minimum(pos, np.float32(NPOS - 1))
    pos_ceil = np.ceil(pos).astype(np.int32)
    pos_floor = np.floor(pos).astype(np.int32)
    logits_int = np.einsum('bhsd,dn->bhsn', query, pos_emb[0, 0])
    lc = np.take_along_axis(logits_int, pos_ceil, axis=-1)
    lf = np.take_along_axis(logits_int, pos_floor, axis=-1)
    w = pos - pos_floor.astype(pos.dtype)
    return (logits + lc * w + lf * (1.0 - w)).astype(np.float32)


def run_on_device(inputs, trace=False):
    """Returns (out [B,H,S,S] f32, flags_ok bool, BassKernelResults)."""
    query = np.asarray(inputs["query"], np.float32)
    attn_logits = np.asarray(inputs["attn_logits"], np.float32)
    pos_emb = np.asarray(inputs["pos_emb"], np.float32)

    attn_f16 = attn_logits.reshape(BH, S, S).astype(np.float16)
    qT_all = np.ascontiguousarray(query.reshape(BH, S, D).transpose(0, 2, 1))
    pe2d = np.ascontiguousarray(pos_emb.reshape(D, NPOS))
    iota_np = np.broadcast_to(
        np.arange(1, WS + 1, dtype=np.int16), (128, WS)).copy()

    nc = _get_program(NT)
    in_maps = _prep_core_inputs(attn_f16, qT_all, pe2d, iota_np)
    res = run_bass_kernel_spmd(nc, in_maps, core_ids=list(range(N_CORES)),
                               trace=trace)
    outs = [res.results[c]["out"] for c in range(N_CORES)]
    fl = [res.results[c]["flags"] for c in range(N_CORES)]
    out = np.concatenate(outs, axis=0).astype(np.float32).reshape(B, H, S, S)
    flags_ok = all(np.all(f >= 0.5) for f in fl)
    return out, flags_ok, res


def kernel(query, attn_logits, mask, pos_emb):
    query = np.asarray(query)
    attn_logits = np.asarray(attn_logits)
    mask = np.asarray(mask)
    pos_emb = np.asarray(pos_emb)
    if not np.all(mask == 1.0):
        return _reference_fallback(
            query.astype(np.float32), attn_logits.astype(np.float32),
            mask.astype(np.float32), pos_emb.astype(np.float32))
    out, flags_ok, _ = run_on_device(
        {"query": query, "attn_logits": attn_logits, "pos_emb": pos_emb})
    if not flags_ok or not np.isfinite(out).all():
        return _reference_fallback(
            query.astype(np.float32), attn_logits.astype(np.float32),
            mask.astype(np.float32), pos_emb.astype(np.float32))
    return out


# revision 18
# speedup vs baseline: 1.0492x; 1.0492x over previous
"""Trainium2 Bass kernel for nn_ContextualPositionEmbedding (B,H,S,D,NPOS = 2,16,2048,64,64).

out[b,h,i,j] = logits + interp(logits_int, pos) where
  gates = sigmoid(attn_logits + log(mask));  pos = clip(reverse-cumsum_j(gates), max 63)
  logits_int = query @ pos_emb;  interp = linear interpolation of logits_int at pos.

Sharding: batch*heads (32 pairs) split 4-per-core across 8 NeuronCores; each core
processes 64 independent [128 x 2048] row-tiles. pos_emb replicated. No collectives.
I/O in fp16 (tolerance is 2e-2 relative; fp16 rounding is far inside budget).

Algorithm per tile (exact where the validity flags pass; a host-side numpy fallback
covers anything else — never triggered for the target workload):
  - pos saturates at 63 for all but the last WS=160 key columns (flag-checked), so
    out = logits + f[r,63] there: one full-row scalar-engine pass with per-row bias.
  - strip [JCUT, 2048): pos = min(reverse-cumsum, 63) comes from ONE reversed
    tensor_tensor_scan with in-scan min-clip. floor(pos) via the +2^23 round trick
    plus an is_gt correction. floor(pos) is a staircase crossing each level 63..1
    exactly once; the last column per level is found with a duplicate-index
    local_scatter (HW is last-wins), then the fp16 table deltas are scattered to
    those columns and prefix-summed (fp32 state): S1[j] = f[fl_j] - f[63] and
    S2[j] = -nd[fl_j] = f[cl_j] - f[fl_j]. out_strip += S1 + w*S2.
"""

import numpy as np
from contextlib import ExitStack

import concourse.bass as bass
import concourse.tile as tile
from concourse import bacc, mybir
from concourse.bass_utils import run_bass_kernel_spmd

F32 = mybir.dt.float32
F16 = mybir.dt.float16
I32 = mybir.dt.int32
I16 = mybir.dt.int16
AF = mybir.ActivationFunctionType
OP = mybir.AluOpType

B, H, S, D, NPOS = 2, 16, 2048, 64, 64
N_CORES = 8
JCUT = 1888
WS = S - JCUT            # 160-wide exact strip
BH = B * H               # 32
BH_PER_CORE = BH // N_CORES   # 4
RB = S // 128            # 16 row-blocks per (b,h)
NT = BH_PER_CORE * RB    # 64 tiles per core
BIG = 8388608.0          # 2^23: float->rne(int) trick
DB = 2 * WS + 2          # dbuf width


def build_program(ntiles=NT, dbg=False):
    nc = bacc.Bacc("TRN2", target_bir_lowering=False, debug=False)
    attn = nc.dram_tensor("attn", [ntiles, 128, S], F16, kind="ExternalInput")
    qT = nc.dram_tensor("qT", [64, ntiles * 128], F32, kind="ExternalInput")
    pe = nc.dram_tensor("pe", [D, NPOS], F32, kind="ExternalInput")
    iota = nc.dram_tensor("iota", [128, WS], I16, kind="ExternalInput")
    out = nc.dram_tensor("out", [ntiles, 128, S], F16, kind="ExternalOutput")
    flags = nc.dram_tensor("flags", [128, ntiles], F32, kind="ExternalOutput")
    if dbg:
        dbg_t = {
            name: nc.dram_tensor(f"dbg_{name}", [ntiles, 128, width], dt,
                                 kind="ExternalOutput")
            for name, width, dt in [
                ("pos", WS, F32), ("w", WS, F32), ("idx1", WS, I16),
                ("idx2", 128, I16), ("dbuf", DB, F16), ("S1", WS, F32),
                ("S2", WS, F32), ("data2", 128, F16), ("f", NPOS, F32),
                ("h", WS, F32),
            ]
        }

    with tile.TileContext(nc) as tc, ExitStack() as ctx:
        const_pool = ctx.enter_context(tc.tile_pool(name="const", bufs=1))
        big_pool = ctx.enter_context(tc.tile_pool(name="big", bufs=8))
        q_pool = ctx.enter_context(tc.tile_pool(name="q", bufs=4))
        psum_pool = ctx.enter_context(tc.tile_pool(name="ps", bufs=4, space="PSUM"))
        tb_pool = ctx.enter_context(tc.tile_pool(name="tb", bufs=8))
        s_pool = ctx.enter_context(tc.tile_pool(name="s", bufs=8))

        pe_sb = const_pool.tile([64, NPOS], F32)
        nc.sync.dma_start(pe_sb[:], pe.ap())
        iota_sb = const_pool.tile([128, WS], I16)
        nc.sync.dma_start(iota_sb[:], iota.ap())
        zi16 = const_pool.tile([128, WS], I16)
        nc.vector.memset(zi16[:], 0)
        c63 = const_pool.tile([128, WS], F32)
        nc.vector.memset(c63[:], 63.0)
        flags_sb = const_pool.tile([128, ntiles], F32)
        # all per-tile query blocks in one DMA: [64, ntiles*128]
        qt_all = const_pool.tile([64, ntiles * 128], F32)
        nc.sync.dma_start(qt_all[:], qT.ap())

        for t in range(ntiles):
            # ---- load (fp16 logits)
            lg = big_pool.tile([128, S], F16, tag="lg")
            nc.sync.dma_start(lg[:], attn.ap()[t])

            # ---- table f = q @ pos_emb  [128, 64]
            fps = psum_pool.tile([128, NPOS], F32, tag="fps")
            nc.tensor.matmul(fps[:], lhsT=qt_all[:, t * 128:(t + 1) * 128],
                             rhs=pe_sb[:], start=True, stop=True)
            f = tb_pool.tile([128, NPOS], F32, tag="f")
            nc.scalar.activation(f[:], fps[:], AF.Copy)

            # ---- fp16 deltas: data2[:,n]=f[n]-f[n+1]; data2[:,64+n]=nd[n+1]-nd[n]
            data2 = tb_pool.tile([128, 128], F16, tag="data2")
            nc.vector.memset(data2[:, 63:64], 0.0)
            nc.vector.tensor_sub(data2[:, 0:63], f[:, 0:63], f[:, 1:64])
            nc.vector.tensor_sub(data2[:, 64:127], data2[:, 1:64], data2[:, 0:63])
            nc.vector.memset(data2[:, 127:128], 0.0)

            # ---- strip: pos = min(reverse-cumsum(sigmoid(lg)), 63), one scan
            gp = s_pool.tile([128, WS], F32, tag="gp")
            nc.scalar.activation(gp[:], lg[:, JCUT:S], AF.Sigmoid)
            pos = s_pool.tile([128, WS], F32, tag="pos")
            nc.vector.tensor_tensor_scan(pos[:, ::-1], gp[:, ::-1], c63[:],
                                         0.0, OP.add, OP.min)

            # fl = floor(pos) via rne (+2^23 trick) + correction; w = pos - fl
            fp_ = s_pool.tile([128, WS], F32, tag="fp")
            nc.vector.tensor_scalar(fp_[:], pos[:], BIG, -BIG, OP.add, OP.add)
            d = s_pool.tile([128, WS], F32, tag="d")
            nc.vector.tensor_sub(d[:], fp_[:], pos[:])
            corr = s_pool.tile([128, WS], F32, tag="corr")
            nc.vector.tensor_scalar(corr[:], d[:], 0.0, None, OP.is_gt)
            w = s_pool.tile([128, WS], F32, tag="w")
            nc.vector.tensor_sub(w[:], corr[:], d[:])
            idx1 = s_pool.tile([128, WS], I16, tag="idx1")
            nc.vector.scalar_tensor_tensor(idx1[:], fp_[:], -1.0, corr[:],
                                           OP.add, OP.subtract)

            # ---- scatter 1 (dup last-wins): last column per level -> idx2[:,0:64]
            idx2 = s_pool.tile([128, 128], I16, tag="idx2")
            nc.gpsimd.local_scatter(idx2[:, 0:64], iota_sb[:], idx1[:],
                                    channels=128, num_elems=64, num_idxs=WS)
            nc.gpsimd.tensor_scalar(idx2[:, 64:127], idx2[:, 0:63], float(WS + 1),
                                    None, OP.add)
            nc.gpsimd.memset(idx2[:, 127:128], -1)
            # ---- scatter 2: fp16 deltas to (drop column + 1), per channel
            dbuf = s_pool.tile([128, DB], F16, tag="dbuf")
            nc.gpsimd.local_scatter(dbuf[:], data2[:], idx2[:],
                                    channels=128, num_elems=DB, num_idxs=128)

            # ---- inclusive prefix sums (fp32 state over fp16)
            S1 = s_pool.tile([128, WS], F32, tag="S1")
            nc.vector.tensor_tensor_scan(S1[:], dbuf[:, 0:WS], zi16[:],
                                         0.0, OP.add, OP.add)
            S2 = s_pool.tile([128, WS], F32, tag="S2")
            nc.vector.tensor_tensor_scan(S2[:], dbuf[:, WS + 1:2 * WS + 1], zi16[:],
                                         0.0, OP.add, OP.add)
            wg2 = s_pool.tile([128, WS], F32, tag="wg2")
            nc.vector.tensor_mul(wg2[:], w[:], S2[:])
            h = s_pool.tile([128, WS], F32, tag="h")
            nc.vector.tensor_add(h[:], S1[:], wg2[:])

            # ---- full-row bias pass: lg += f[:,63]  (in-place on lg)
            nc.scalar.activation(lg[:], lg[:], AF.Identity,
                                 bias=f[:, 63:64], scale=1.0)
            # ---- strip combine
            nc.gpsimd.tensor_add(lg[:, JCUT:S], lg[:, JCUT:S], h[:])

            # ---- flags: pos[0] == 63 (saturated) AND all levels 1..63 deposited
            posok = s_pool.tile([128, 1], F32, tag="posok")
            nc.vector.tensor_scalar(posok[:], pos[:, 0:1], 63.0, None, OP.is_ge)
            rmin = s_pool.tile([128, 1], I16, tag="rmin")
            nc.vector.tensor_reduce(rmin[:], idx2[:, 0:63], mybir.AxisListType.X,
                                    OP.min)
            nc.vector.scalar_tensor_tensor(flags_sb[:, t:t + 1], rmin[:], 0.5,
                                           posok[:], OP.is_ge, OP.mult)

            if dbg:
                for name, ap_ in [("pos", pos[:]), ("w", w[:]), ("idx1", idx1[:]),
                                  ("idx2", idx2[:]), ("dbuf", dbuf[:]),
                                  ("S1", S1[:]), ("S2", S2[:]),
                                  ("data2", data2[:]), ("f", f[:]), ("h", h[:])]:
                    nc.sync.dma_start(dbg_t[name].ap()[t], ap_)

            # ---- store (Pool SWDGE queue: triggered right after the strip add,
            # same engine, so it never head-blocks another engine's sequencer)
            nc.gpsimd.dma_start(out.ap()[t], lg[:])

        nc.sync.dma_start(flags.ap(), flags_sb[:])

    nc.compile()
    return nc


_PROG_CACHE = {}


def _get_program(ntiles=NT):
    if ntiles not in _PROG_CACHE:
        _PROG_CACHE[ntiles] = build_program(ntiles)
    return _PROG_CACHE[ntiles]


def _prep_core_inputs(attn_f16, qT_all, pe2d, iota_np):
    """attn_f16: [BH, S, S] fp16; qT_all: [BH, D, S]. Returns list of 8 in_maps."""
    in_maps = []
    for c in range(N_CORES):
        sl = slice(c * BH_PER_CORE, (c + 1) * BH_PER_CORE)
        a = attn_f16[sl].reshape(NT, 128, S)
        q = np.ascontiguousarray(
            qT_all[sl].reshape(BH_PER_CORE, D, RB, 128).transpose(1, 0, 2, 3)
        ).reshape(D, NT * 128)
        in_maps.append({"attn": np.ascontiguousarray(a), "qT": q,
                        "pe": pe2d, "iota": iota_np})
    return in_maps


def _reference_fallback(query, attn_logits, mask, pos_emb):
    logits = attn_logits + np.log(mask)
    gates = 1.0 / (1.0 + np.exp(-logits))
    pos = np.cumsum(gates[..., ::-1], axis=-1)[..., ::-1]
    pos = np.minimum(pos, np.float32(NPOS - 1))
    pos_ceil = np.ceil(pos).astype(np.int32)
    pos_floor = np.floor(pos).astype(np.int32)
    logits_int = np.einsum('bhsd,dn->bhsn', query, pos_emb[0, 0])
    lc = np.take_along_axis(logits_int, pos_ceil, axis=-1)
    lf = np.take_along_axis(logits_int, pos_floor, axis=-1)
    w = pos - pos_floor.astype(pos.dtype)
    return (logits + lc * w + lf * (1.0 - w)).astype(np.float32)


def run_on_device(inputs, trace=False):
    """Returns (out [B,H,S,S] f32, flags_ok bool, BassKernelResults)."""
    query = np.asarray(inputs["query"], np.float32)
    attn_logits = np.asarray(inputs["attn_logits"], np.float32)
    pos_emb = np.asarray(inputs["pos_emb"], np.float32)

    attn_f16 = attn_logits.reshape(BH, S, S).astype(np.float16)
    qT_all = np.ascontiguousarray(query.reshape(BH, S, D).transpose(0, 2, 1))
    pe2d = np.ascontiguousarray(pos_emb.reshape(D, NPOS))
    iota_np = np.broadcast_to(
        np.arange(1, WS + 1, dtype=np.int16), (128, WS)).copy()

    nc = _get_program(NT)
    in_maps = _prep_core_inputs(attn_f16, qT_all, pe2d, iota_np)
    res = run_bass_kernel_spmd(nc, in_maps, core_ids=list(range(N_CORES)),
                               trace=trace)
    outs = [res.results[c]["out"] for c in range(N_CORES)]
    fl = [res.results[c]["flags"] for c in range(N_CORES)]
    out = np.concatenate(outs, axis=0).astype(np.float32).reshape(B, H, S, S)
    flags_ok = all(np.all(f >= 0.5) for f in fl)
    return out, flags_ok, res


def kernel(query, attn_logits, mask, pos_emb):
    query = np.asarray(query)
    attn_logits = np.asarray(attn_logits)
    mask = np.asarray(mask)
    pos_emb = np.asarray(pos_emb)
    if not np.all(mask == 1.0):
        return _reference_fallback(
            query.astype(np.float32), attn_logits.astype(np.float32),
            mask.astype(np.float32), pos_emb.astype(np.float32))
    out, flags_ok, _ = run_on_device(
        {"query": query, "attn_logits": attn_logits, "pos_emb": pos_emb})
    if not flags_ok or not np.isfinite(out).all():
        return _reference_fallback(
            query.astype(np.float32), attn_logits.astype(np.float32),
            mask.astype(np.float32), pos_emb.astype(np.float32))
    return out
